# revision 1
# baseline (speedup 1.0000x reference)
"""Trainium2 Bass kernel for nn_BigAttention (weight-norm MLP + softmax-over-k).

Math (per the reference):
    W1e = g1 * W1 / ||W1||_F          [1024, 3072]
    W2e = g2 * W2 / ||W2||_F          [1, 1024]
    hv  = v @ W1e[:, :2048].T         [B,K,N,1024]
    hq  = q @ W1e[:, 2048:].T         [B,K,1024]
    joint  = relu(hv + hq + b1)
    logits = joint @ W2e.T  (+ b2, which cancels in the softmax over k)
    out = softmax(logits, axis=K)     [B,K,N,1]

Sharding: data-parallel over batch, 8 batches per core; weights replicated.

Per-core device program (rows r = (b_local, k, n) flattened, R = 8*12*36 = 3456):
  - hq[96, 1024] via PE (q^T chunks stationary), b1 folded in as a K=1 matmul.
  - main: per 128-row tile, PSUM[row, hidden 1024] accumulates 16 v^T-chunk
    matmuls (float32r: 1 cycle/row vs 4 for fp32) plus ONE one-hot matmul
    that adds hq[bk(row), :] (one-hot selection stationary, hq moving).
  - epilogue per tile: one DVE scalar_tensor_tensor computes
    (PSUM max 0) * w2_broadcast with accum_out = per-row sum = the logit.
  - softmax over k: logits go [128, 27] -> StreamTranspose -> linear DRAM ->
    [96 (b,k), 36 n] SBUF; exp on ACT; the per-(b,n) sum and its broadcast
    back over k are two tiny one-hot matmuls on the PE; final scale on DVE;
    one strided DMA writes the [8,12,36,1] output slice.

All heavy inputs are host-repacked "partition-major" so every big DMA is 128
contiguous runs (one per partition) instead of thousands of thin descriptors.
Weight DMAs ride the scalar-engine HWDGE ring, v DMAs the sync ring, tiny
constants the gpsimd SWDGE path, so descriptor generation overlaps.
"""

import ml_dtypes
import numpy as np

import concourse.bacc as bacc
import concourse.mybir as mybir
import concourse.tile as tile
from concourse.bass_utils import run_bass_kernel_spmd

F32 = mybir.dt.float32
NCORES = 8
B, K, N = 64, 12, 36
VD, QD, HID = 2048, 1024, 1024
BL = B // NCORES              # local batches per core
R = BL * K * N                # 3456 rows per core
BK = BL * K                   # 96 (b,k) groups per core
CC = VD // 128                # 16 contraction chunks over v-dim
QC = QD // 128                # 8 contraction chunks over q-dim
RC = 384                      # rows per DMA chunk (9 chunks)
NCH = R // RC
RT = 128                      # rows per PSUM tile
NT = RC // RT
NRT = R // RT                 # 27 row tiles
VSPLIT = 8                    # v-chunk DMA granularity (cc chunks per DMA)

_NC_CACHE = None

# fp32 matmul runs at 4 cycles/row on the PE (decomposed into 2 half-rate
# passes); float32r (same 4-byte data, relaxed-precision multiply) streams at
# 1 cycle/row when the moving free dim is >= 256.
MM_DT = mybir.dt.float32r
# big streaming tensors go bf16: same PE rate (1 cycle/row), half the HBM/
# upload bytes, and FWL fast weight loads for the bf16 stationary operand.
BF16 = mybir.dt.bfloat16


def _build_nc():
    nc = bacc.Bacc("TRN2", target_bir_lowering=False, debug=False,
                   num_devices=NCORES)

    def mm(out, lhsT, rhs, **kw):
        nc.tensor.matmul(out, lhsT, rhs, **kw)

    w1vt = nc.dram_tensor("w1vt", [128, CC, HID], BF16, kind="ExternalInput").ap()
    # qt and W1q^T packed along the free dim: [:, cq, 0:96]=q^T, [:, cq, 96:1120]=W1q^T
    qtwq = nc.dram_tensor("qtwq", [128, QC, BK + HID], BF16, kind="ExternalInput").ap()
    # fp32r one-hot row-selection matrix for the hq-add closers
    # (must match hq_s dtype: walrus rejects bf16 x fp32r matmuls)
    oneh_d = nc.dram_tensor("oneh", [BK, R], MM_DT, kind="ExternalInput").ap()
    # fp32 constants pack: [:, 0:1024]=w2 bcast, [0:96, 1024:1032]=selb,
    # [0:8, 1032:1128]=selb^T, [0:96, 1128:2152]=b1 replicated
    packf = nc.dram_tensor("packf", [128, HID + BL + BK + HID], F32, kind="ExternalInput").ap()
    # v is split: the first two chunks ride with the weights at the front of
    # the upload order; the bulk uploads last, hidden under early compute.
    vth = nc.dram_tensor("vth", [2, 128, CC, RC], BF16, kind="ExternalInput").ap()
    vtr = nc.dram_tensor("vtr", [NCH - 2, 128, CC, RC], BF16, kind="ExternalInput").ap()
    out = nc.dram_tensor("out", [BL, K, N, 1], F32, kind="ExternalOutput").ap()

    MAX = mybir.AluOpType.max
    MULT = mybir.AluOpType.mult
    BYPASS = mybir.AluOpType.bypass
    ADD = mybir.AluOpType.add

    with tile.TileContext(nc) as tc:
        with tc.tile_pool(name="const", bufs=1) as cpool, \
             tc.tile_pool(name="wv", bufs=1) as wvpool, \
             tc.tile_pool(name="vtp", bufs=2) as vtpool, \
             tc.tile_pool(name="work", bufs=3) as work, \
             tc.tile_pool(name="small", bufs=1) as small, \
             tc.tile_pool(name="dram", bufs=1, space="DRAM") as dpool, \
             tc.tile_pool(name="psum", bufs=4, space="PSUM") as pspool:

            # ---- startup set as ~9 fat DMAs (the Tile runtime can only track
            # ~8 outstanding DMA completions; many small DMAs serialize and
            # starve the PE). Issue order matches consumption order.
            packf_s = cpool.tile([128, HID + BL + BK + HID], F32)
            nc.sync.dma_start(out=packf_s, in_=packf)

            def vt_chunk_tiles(ch):
                src_ap = vth[ch] if ch < 2 else vtr[ch - 2]
                tiles = []
                for j in range(CC // VSPLIT):
                    t = vtpool.tile([128, VSPLIT, RC], BF16, tag=f"vt{j}")
                    nc.sync.dma_start(
                        out=t, in_=src_ap[:, j * VSPLIT:(j + 1) * VSPLIT, :])
                    tiles.append(t)
                return tiles

            vt_cur = vt_chunk_tiles(0)

            qtwq_s = cpool.tile([128, QC, BK + HID], BF16)
            nc.sync.dma_start(out=qtwq_s, in_=qtwq)

            vt_next = vt_chunk_tiles(1)

            oneh_s = cpool.tile([BK, R], MM_DT)
            nc.sync.dma_start(out=oneh_s, in_=oneh_d)

            WG = 4  # wv group size (cc chunks per DMA)
            wv_g = []
            for j in range(CC // WG):
                t = wvpool.tile([128, WG, HID], BF16, tag=f"wvg{j}")
                nc.scalar.dma_start(out=t, in_=w1vt[:, j * WG:(j + 1) * WG, :])
                wv_g.append(t)



            w2b_s = packf_s[:, 0:HID]
            selb_s = packf_s[0:BK, HID:HID + BL]
            selbt_s = packf_s[0:BL, HID + BL:HID + BL + BK]
            b1b_s = packf_s[0:BK, HID + BL + BK:HID + BL + BK + HID]

            # per-row logits, laid out [p, rt] with row = rt*128 + p, split
            # into two tiles so the first half's DRAM flush hides under the
            # main loop. 32 columns (StreamTranspose needs 32x32 blocks).
            NRT_A = 18   # 18*128 rows = 64 (b,k) groups — a 32-aligned bk split
            ls_a = cpool.tile([128, 32], F32)
            nc.vector.memset(ls_a, 0.0)
            ls_b = cpool.tile([128, 32], F32)
            nc.vector.memset(ls_b, 0.0)
            lg = dpool.tile([R], F32)
            lg2 = lg.rearrange("(t p) -> t p", t=NRT, p=128)

            def flush_logits(ls, ls_t_name, t0, t1):
                # ls[p, t - t0] holds L[t*128 + p] for t in [t0, t1)
                ls_t = cpool.tile([128, 32], F32, name=ls_t_name)
                nc.vector.transpose(ls_t, ls)
                for i in range(4):
                    eng = nc.sync if i % 2 == 0 else nc.scalar
                    eng.dma_start(
                        out=lg2[t0:t1, 32 * i:32 * i + 32],
                        in_=ls_t[32 * i:32 * i + (t1 - t0), :])

            hq_s = cpool.tile([BK, HID], MM_DT)
            s96 = small.tile([BK, N], F32)
            e96 = small.tile([BK, N], F32)
            sums_ps = pspool.tile([BL, N], F32, tag="sm", bufs=2)

            def emit_vmms(t, ps):
                for cc in range(CC):
                    lhsT = vt_cur[cc // VSPLIT][:, cc % VSPLIT:cc % VSPLIT + 1,
                                                t * RT:(t + 1) * RT]
                    wvc = wv_g[cc // WG][:, cc % WG:cc % WG + 1, :]
                    mm(ps[:, 0:512], lhsT, wvc[:, :, 0:512],
                       start=(cc == 0), stop=False)
                    mm(ps[:, 512:1024], lhsT, wvc[:, :, 512:1024],
                       start=(cc == 0), stop=False)

            def emit_closer(rt, ps):
                oh = oneh_s[:, rt * RT:(rt + 1) * RT]
                mm(ps[:, 0:512], oh, hq_s[:, 0:512], start=False, stop=True)
                mm(ps[:, 512:1024], oh, hq_s[:, 512:1024], start=False, stop=True)
                relu_w2 = work.tile([128, HID], F32, tag="relu_w2")
                ls, col = (ls_a, rt) if rt < NRT_A else (ls_b, rt - NRT_A)
                nc.vector.scalar_tensor_tensor(
                    out=relu_w2, in0=ps, scalar=0.0, in1=w2b_s,
                    op0=MAX, op1=MULT,
                    accum_out=ls[:, col:col + 1])
                if rt == NRT_A - 1:
                    # flush + start the softmax head for bk rows 0:64 while
                    # the main loop still runs
                    flush_logits(ls_a, "ls_ta", 0, NRT_A)
                    nc.sync.dma_start(
                        out=s96[0:64, :],
                        in_=lg.rearrange("(bk n) -> bk n", n=N)[0:64, :])
                    nc.scalar.activation(e96[0:64, :], s96[0:64, :],
                                         mybir.ActivationFunctionType.Exp)
                    mm(sums_ps, selb_s[0:64, :], e96[0:64, :],
                       start=True, stop=False)

            # ---- chunk 0: v-matmuls for tiles 0..2 first, then hq (its DMAs
            # arrive under the v work), then the deferred closers.
            ps0 = []
            for t in range(NT):
                ps = pspool.tile([128, HID], F32, tag="ps", bufs=3)
                emit_vmms(t, ps)
                ps0.append(ps)

            # hq[bk, h] = q @ W1q^T + b1 (PSUM halves live in the small pool)
            hq_ps = [pspool.tile([BK, 512], F32, tag="sm", bufs=2,
                                 name=f"hq_ps{i}") for i in range(2)]
            for half in range(2):
                hs = slice(half * 512, (half + 1) * 512)
                for cq in range(QC):
                    mm(hq_ps[half],
                       qtwq_s[:, cq:cq + 1, 0:BK],
                       qtwq_s[:, cq:cq + 1, BK + half * 512:BK + (half + 1) * 512],
                       start=(cq == 0), stop=(cq == QC - 1))
                # psum -> SBUF with the b1 row added (b1b is b1 replicated
                # across the 96 partitions host-side)
                nc.vector.scalar_tensor_tensor(
                    out=hq_s[:, hs], in0=hq_ps[half], scalar=0.0,
                    in1=b1b_s[:, half * 512:(half + 1) * 512],
                    op0=BYPASS, op1=ADD)

            for t in range(NT):
                emit_closer(t, ps0[t])
            vt_cur = vt_next

            # ---- chunks 1..8
            for ch in range(1, NCH):
                if ch + 1 < NCH:
                    vt_next = vt_chunk_tiles(ch + 1)
                for t in range(NT):
                    rt = ch * NT + t
                    ps = pspool.tile([128, HID], F32, tag="ps", bufs=3)
                    emit_vmms(t, ps)
                    emit_closer(rt, ps)
                vt_cur = vt_next

            # ---- flush remaining logits, finish the softmax
            flush_logits(ls_b, "ls_tb", NRT_A, NRT)
            nc.sync.dma_start(
                out=s96[64:BK, :],
                in_=lg.rearrange("(bk n) -> bk n", n=N)[64:BK, :])
            nc.scalar.activation(e96[64:BK, :], s96[64:BK, :],
                                 mybir.ActivationFunctionType.Exp)
            mm(sums_ps, selb_s[64:BK, :], e96[64:BK, :],
               start=False, stop=True)
            rcp = small.tile([BL, N], F32)
            nc.vector.reciprocal(rcp, sums_ps)
            rexp_ps = pspool.tile([BK, N], F32, tag="sm", bufs=2)
            mm(rexp_ps, selbt_s, rcp, start=True, stop=True)
            w96 = small.tile([BK, N], F32)
            nc.vector.scalar_tensor_tensor(
                out=w96, in0=e96, scalar=0.0, in1=rexp_ps,
                op0=BYPASS, op1=MULT)
            nc.sync.dma_start(
                out=out.rearrange("b k n o -> (b k) (n o)"), in_=w96)

    nc.compile()
    return nc


def _get_nc():
    global _NC_CACHE
    if _NC_CACHE is None:
        _NC_CACHE = _build_nc()
    return _NC_CACHE


def _prepare_in_maps(inputs):
    v = np.asarray(inputs["v"], dtype=np.float32)
    q = np.asarray(inputs["q"], dtype=np.float32)
    W1 = np.asarray(inputs["W1"], dtype=np.float32)
    g1 = np.float64(np.asarray(inputs["g1"]))
    b1 = np.asarray(inputs["b1"], dtype=np.float32)
    W2 = np.asarray(inputs["W2"], dtype=np.float32)
    g2 = np.float64(np.asarray(inputs["g2"]))
    # b2 is a scalar added to every logit -> cancels in softmax over k.

    W1e = ((g1 / np.linalg.norm(W1.astype(np.float64))) * W1).astype(np.float32)
    W2e = ((g2 / np.linalg.norm(W2.astype(np.float64))) * W2).astype(np.float32)

    BF = ml_dtypes.bfloat16
    # partition-major repacks: [..., 128 p, chunk, inner]
    w1vt = np.ascontiguousarray(                       # [128, 16, 1024]
        W1e[:, :VD].T.reshape(CC, 128, HID).transpose(1, 0, 2)).astype(BF)
    w1qt = W1e[:, VD:].T.reshape(QC, 128, HID).transpose(1, 0, 2)  # [128, 8, 1024]
    r = np.arange(R)
    oneh = (np.arange(BK)[:, None] == (r // N)[None, :]).astype(np.float32)
    selb = (np.arange(BL)[None, :] == (np.arange(BK) // K)[:, None]).astype(np.float32)

    packf = np.zeros((128, HID + BL + BK + HID), dtype=np.float32)
    packf[:, 0:HID] = W2e.reshape(1, HID)
    packf[0:BK, HID:HID + BL] = selb
    packf[0:BL, HID + BL:HID + BL + BK] = selb.T
    packf[0:BK, HID + BL + BK:HID + BL + BK + HID] = b1.reshape(1, HID)

    shared = dict(w1vt=w1vt, oneh=oneh, packf=packf)
    in_maps = []
    for c in range(NCORES):
        vl = v[c * BL:(c + 1) * BL].reshape(R, VD)
        # vt[ch, p, cc, r_in_chunk] = v[ch*RC + r, cc*128 + p]
        vt4 = np.ascontiguousarray(
            vl.T.reshape(CC, 128, NCH, RC).transpose(2, 1, 0, 3)).astype(BF)
        ql = q[c * BL:(c + 1) * BL].reshape(BK, QD)
        qt3 = ql.T.reshape(QC, 128, BK).transpose(1, 0, 2)   # [128, 8, 96]
        qtwq = np.concatenate([qt3, w1qt], axis=2).astype(BF)  # [128, 8, 1120]
        in_maps.append(dict(vth=np.ascontiguousarray(vt4[:2]),
                            vtr=np.ascontiguousarray(vt4[2:]),
                            qtwq=np.ascontiguousarray(qtwq), **shared))
    return in_maps


def kernel(**inputs) -> np.ndarray:
    in_maps = _prepare_in_maps(inputs)
    nc = _get_nc()
    res = run_bass_kernel_spmd(nc, in_maps, list(range(NCORES)))
    outs = [res.results[c]["out"].reshape(BL, K, N, 1) for c in range(NCORES)]
    return np.concatenate(outs, axis=0)



# revision 2
# speedup vs baseline: 1.7037x; 1.7037x over previous
"""Trainium2 Bass kernel for nn_BigAttention (weight-norm MLP + softmax-over-k).

Math (per the reference):
    W1e = g1 * W1 / ||W1||_F          [1024, 3072]
    W2e = g2 * W2 / ||W2||_F          [1, 1024]
    hv  = v @ W1e[:, :2048].T         [B,K,N,1024]
    hq  = q @ W1e[:, 2048:].T         [B,K,1024]
    joint  = relu(hv + hq + b1)
    logits = joint @ W2e.T  (+ b2, which cancels in the softmax over k)
    out = softmax(logits, axis=K)     [B,K,N,1]

Sharding: data-parallel over batch, 8 batches per core; weights replicated.

Per-core device program (rows r = (b_local, k, n) flattened, R = 8*12*36 = 3456):
  - hq[96, 1024] via PE (q^T chunks stationary), b1 folded in as a K=1 matmul.
  - main: per 128-row tile, PSUM[row, hidden 1024] accumulates 16 v^T-chunk
    matmuls (float32r: 1 cycle/row vs 4 for fp32) plus ONE one-hot matmul
    that adds hq[bk(row), :] (one-hot selection stationary, hq moving).
  - epilogue per tile: one DVE scalar_tensor_tensor computes
    (PSUM max 0) * w2_broadcast with accum_out = per-row sum = the logit.
  - softmax over k: logits go [128, 27] -> StreamTranspose -> linear DRAM ->
    [96 (b,k), 36 n] SBUF; exp on ACT; the per-(b,n) sum and its broadcast
    back over k are two tiny one-hot matmuls on the PE; final scale on DVE;
    one strided DMA writes the [8,12,36,1] output slice.

All heavy inputs are host-repacked "partition-major" so every big DMA is 128
contiguous runs (one per partition) instead of thousands of thin descriptors.
Weight DMAs ride the scalar-engine HWDGE ring, v DMAs the sync ring, tiny
constants the gpsimd SWDGE path, so descriptor generation overlaps.
"""

import ml_dtypes
import numpy as np

import concourse.bacc as bacc
import concourse.mybir as mybir
import concourse.tile as tile
from concourse.bass_utils import run_bass_kernel_spmd

F32 = mybir.dt.float32
NCORES = 8
B, K, N = 64, 12, 36
VD, QD, HID = 2048, 1024, 1024
BL = B // NCORES              # local batches per core
R = BL * K * N                # 3456 rows per core
BK = BL * K                   # 96 (b,k) groups per core
CC = VD // 128                # 16 contraction chunks over v-dim
QC = QD // 128                # 8 contraction chunks over q-dim
RC = 384                      # rows per DMA chunk (9 chunks)
NCH = R // RC
RT = 128                      # rows per PSUM tile
NT = RC // RT
NRT = R // RT                 # 27 row tiles
VSPLIT = 8                    # v-chunk DMA granularity (cc chunks per DMA)

_NC_CACHE = None

# fp32 matmul runs at 4 cycles/row on the PE (decomposed into 2 half-rate
# passes); float32r (same 4-byte data, relaxed-precision multiply) streams at
# 1 cycle/row when the moving free dim is >= 256.
MM_DT = mybir.dt.float32r
# big streaming tensors go bf16: same PE rate (1 cycle/row), half the HBM/
# upload bytes, and FWL fast weight loads for the bf16 stationary operand.
BF16 = mybir.dt.bfloat16
# the dominant v-matmuls go fp8 e4m3 with perf_mode=DoubleRow: two 128-deep
# contraction chunks fuse into one instruction streaming 2 cols/cycle.
# W1v is scaled by S1 host-side so its entries (std ~5.6e-4) land in e4m3's
# representable band; W2 carries 1/S1 (relu is scale-equivariant).
FP8 = mybir.dt.float8e4
S1 = 2048.0


def _build_nc():
    nc = bacc.Bacc("TRN2", target_bir_lowering=False, debug=False,
                   num_devices=NCORES)

    def mm(out, lhsT, rhs, **kw):
        nc.tensor.matmul(out, lhsT, rhs, **kw)

    w1vt = nc.dram_tensor("w1vt", [128, CC, HID], FP8, kind="ExternalInput").ap()
    # qt and W1q^T packed along the free dim: [:, cq, 0:96]=q^T, [:, cq, 96:1120]=W1q^T
    qtwq = nc.dram_tensor("qtwq", [128, QC, BK + HID], BF16, kind="ExternalInput").ap()
    # fp32r one-hot row-selection matrix for the hq-add closers
    # (must match hq_s dtype: walrus rejects bf16 x fp32r matmuls)
    oneh_d = nc.dram_tensor("oneh", [BK, R], MM_DT, kind="ExternalInput").ap()
    # fp32 constants pack: [:, 0:1024]=w2 bcast, [0:96, 1024:1032]=selb,
    # [0:8, 1032:1128]=selb^T, [0:96, 1128:2152]=b1 replicated
    packf = nc.dram_tensor("packf", [128, HID + BL + BK + HID], F32, kind="ExternalInput").ap()
    # v is split: the first two chunks ride with the weights at the front of
    # the upload order; the bulk uploads last, hidden under early compute.
    vth = nc.dram_tensor("vth", [2, 128, CC, RC], FP8, kind="ExternalInput").ap()
    vtr = nc.dram_tensor("vtr", [NCH - 2, 128, CC, RC], FP8, kind="ExternalInput").ap()
    out = nc.dram_tensor("out", [BL, K, N, 1], F32, kind="ExternalOutput").ap()

    MAX = mybir.AluOpType.max
    MULT = mybir.AluOpType.mult
    BYPASS = mybir.AluOpType.bypass
    ADD = mybir.AluOpType.add

    with tile.TileContext(nc) as tc:
        with tc.tile_pool(name="const", bufs=1) as cpool, \
             tc.tile_pool(name="wv", bufs=1) as wvpool, \
             tc.tile_pool(name="vtp", bufs=2) as vtpool, \
             tc.tile_pool(name="work", bufs=3) as work, \
             tc.tile_pool(name="small", bufs=1) as small, \
             tc.tile_pool(name="dram", bufs=1, space="DRAM") as dpool, \
             tc.tile_pool(name="psum", bufs=4, space="PSUM") as pspool:

            # ---- startup set as ~9 fat DMAs (the Tile runtime can only track
            # ~8 outstanding DMA completions; many small DMAs serialize and
            # starve the PE). Issue order matches consumption order.
            packf_s = cpool.tile([128, HID + BL + BK + HID], F32)
            nc.sync.dma_start(out=packf_s, in_=packf)

            def vt_chunk_tiles(ch):
                src_ap = vth[ch] if ch < 2 else vtr[ch - 2]
                tiles = []
                for j in range(CC // VSPLIT):
                    t = vtpool.tile([128, VSPLIT, RC], FP8, tag=f"vt{j}")
                    nc.sync.dma_start(
                        out=t, in_=src_ap[:, j * VSPLIT:(j + 1) * VSPLIT, :])
                    tiles.append(t)
                return tiles

            vt_cur = vt_chunk_tiles(0)

            qtwq_s = cpool.tile([128, QC, BK + HID], BF16)
            nc.sync.dma_start(out=qtwq_s, in_=qtwq)

            vt_next = vt_chunk_tiles(1)

            oneh_s = cpool.tile([BK, R], MM_DT)
            nc.sync.dma_start(out=oneh_s, in_=oneh_d)

            WG = 4  # wv group size (cc chunks per DMA)
            wv_g = []
            for j in range(CC // WG):
                t = wvpool.tile([128, WG, HID], FP8, tag=f"wvg{j}")
                nc.scalar.dma_start(out=t, in_=w1vt[:, j * WG:(j + 1) * WG, :])
                wv_g.append(t)



            w2b_s = packf_s[:, 0:HID]
            selb_s = packf_s[0:BK, HID:HID + BL]
            selbt_s = packf_s[0:BL, HID + BL:HID + BL + BK]
            b1b_s = packf_s[0:BK, HID + BL + BK:HID + BL + BK + HID]

            # per-row logits, laid out [p, rt] with row = rt*128 + p, split
            # into two tiles so the first half's DRAM flush hides under the
            # main loop. 32 columns (StreamTranspose needs 32x32 blocks).
            NRT_A = 18   # 18*128 rows = 64 (b,k) groups — a 32-aligned bk split
            ls_a = cpool.tile([128, 32], F32)
            nc.vector.memset(ls_a, 0.0)
            ls_b = cpool.tile([128, 32], F32)
            nc.vector.memset(ls_b, 0.0)
            lg = dpool.tile([R], F32)
            lg2 = lg.rearrange("(t p) -> t p", t=NRT, p=128)

            def flush_logits(ls, ls_t_name, t0, t1):
                # ls[p, t - t0] holds L[t*128 + p] for t in [t0, t1)
                ls_t = cpool.tile([128, 32], F32, name=ls_t_name)
                nc.vector.transpose(ls_t, ls)
                for i in range(4):
                    eng = nc.sync if i % 2 == 0 else nc.scalar
                    eng.dma_start(
                        out=lg2[t0:t1, 32 * i:32 * i + 32],
                        in_=ls_t[32 * i:32 * i + (t1 - t0), :])

            hq_s = cpool.tile([BK, HID], MM_DT)
            s96 = small.tile([BK, N], F32)
            e96 = small.tile([BK, N], F32)
            sums_ps = pspool.tile([BL, N], F32, tag="sm", bufs=2)

            DR = mybir.MatmulPerfMode.DoubleRow

            def emit_vmms(t, ps):
                # fp8 DoubleRow: each matmul contracts TWO 128-deep v chunks
                # (lhsT [128, 2, 128 rows], rhs [128, 2, 512]) at 2 cols/cycle.
                for cc in range(0, CC, 2):
                    lhsT = vt_cur[cc // VSPLIT][:, cc % VSPLIT:cc % VSPLIT + 2,
                                                t * RT:(t + 1) * RT]
                    wvc = wv_g[cc // WG][:, cc % WG:cc % WG + 2, :]
                    mm(ps[:, 0:512], lhsT, wvc[:, :, 0:512],
                       start=(cc == 0), stop=False, perf_mode=DR)
                    mm(ps[:, 512:1024], lhsT, wvc[:, :, 512:1024],
                       start=(cc == 0), stop=False, perf_mode=DR)

            def emit_closer(rt, ps):
                oh = oneh_s[:, rt * RT:(rt + 1) * RT]
                mm(ps[:, 0:512], oh, hq_s[:, 0:512], start=False, stop=True)
                mm(ps[:, 512:1024], oh, hq_s[:, 512:1024], start=False, stop=True)
                relu_w2 = work.tile([128, HID], F32, tag="relu_w2")
                ls, col = (ls_a, rt) if rt < NRT_A else (ls_b, rt - NRT_A)
                nc.vector.scalar_tensor_tensor(
                    out=relu_w2, in0=ps, scalar=0.0, in1=w2b_s,
                    op0=MAX, op1=MULT,
                    accum_out=ls[:, col:col + 1])
                if rt == NRT_A - 1:
                    # flush + start the softmax head for bk rows 0:64 while
                    # the main loop still runs
                    flush_logits(ls_a, "ls_ta", 0, NRT_A)
                    nc.sync.dma_start(
                        out=s96[0:64, :],
                        in_=lg.rearrange("(bk n) -> bk n", n=N)[0:64, :])
                    nc.scalar.activation(e96[0:64, :], s96[0:64, :],
                                         mybir.ActivationFunctionType.Exp)
                    mm(sums_ps, selb_s[0:64, :], e96[0:64, :],
                       start=True, stop=False)

            # ---- chunk 0: v-matmuls for tiles 0..2 first, then hq (its DMAs
            # arrive under the v work), then the deferred closers.
            ps0 = []
            for t in range(NT):
                ps = pspool.tile([128, HID], F32, tag="ps", bufs=3)
                emit_vmms(t, ps)
                ps0.append(ps)

            # hq[bk, h] = q @ W1q^T + b1 (PSUM halves live in the small pool)
            hq_ps = [pspool.tile([BK, 512], F32, tag="sm", bufs=2,
                                 name=f"hq_ps{i}") for i in range(2)]
            for half in range(2):
                hs = slice(half * 512, (half + 1) * 512)
                for cq in range(QC):
                    mm(hq_ps[half],
                       qtwq_s[:, cq:cq + 1, 0:BK],
                       qtwq_s[:, cq:cq + 1, BK + half * 512:BK + (half + 1) * 512],
                       start=(cq == 0), stop=(cq == QC - 1))
                # psum -> SBUF with the b1 row added (b1b is b1 replicated
                # across the 96 partitions host-side)
                nc.vector.scalar_tensor_tensor(
                    out=hq_s[:, hs], in0=hq_ps[half], scalar=0.0,
                    in1=b1b_s[:, half * 512:(half + 1) * 512],
                    op0=BYPASS, op1=ADD)

            for t in range(NT):
                emit_closer(t, ps0[t])
            vt_cur = vt_next

            # ---- chunks 1..8
            for ch in range(1, NCH):
                if ch + 1 < NCH:
                    vt_next = vt_chunk_tiles(ch + 1)
                for t in range(NT):
                    rt = ch * NT + t
                    ps = pspool.tile([128, HID], F32, tag="ps", bufs=3)
                    emit_vmms(t, ps)
                    emit_closer(rt, ps)
                vt_cur = vt_next

            # ---- flush remaining logits, finish the softmax
            flush_logits(ls_b, "ls_tb", NRT_A, NRT)
            nc.sync.dma_start(
                out=s96[64:BK, :],
                in_=lg.rearrange("(bk n) -> bk n", n=N)[64:BK, :])
            nc.scalar.activation(e96[64:BK, :], s96[64:BK, :],
                                 mybir.ActivationFunctionType.Exp)
            mm(sums_ps, selb_s[64:BK, :], e96[64:BK, :],
               start=False, stop=True)
            rcp = small.tile([BL, N], F32)
            nc.vector.reciprocal(rcp, sums_ps)
            rexp_ps = pspool.tile([BK, N], F32, tag="sm", bufs=2)
            mm(rexp_ps, selbt_s, rcp, start=True, stop=True)
            w96 = small.tile([BK, N], F32)
            nc.vector.scalar_tensor_tensor(
                out=w96, in0=e96, scalar=0.0, in1=rexp_ps,
                op0=BYPASS, op1=MULT)
            nc.sync.dma_start(
                out=out.rearrange("b k n o -> (b k) (n o)"), in_=w96)

    nc.compile()
    return nc


def _get_nc():
    global _NC_CACHE
    if _NC_CACHE is None:
        _NC_CACHE = _build_nc()
    return _NC_CACHE


def _prepare_in_maps(inputs):
    v = np.asarray(inputs["v"], dtype=np.float32)
    q = np.asarray(inputs["q"], dtype=np.float32)
    W1 = np.asarray(inputs["W1"], dtype=np.float32)
    g1 = np.float64(np.asarray(inputs["g1"]))
    b1 = np.asarray(inputs["b1"], dtype=np.float32)
    W2 = np.asarray(inputs["W2"], dtype=np.float32)
    g2 = np.float64(np.asarray(inputs["g2"]))
    # b2 is a scalar added to every logit -> cancels in softmax over k.

    W1e = ((g1 / np.linalg.norm(W1.astype(np.float64))) * W1).astype(np.float32)
    W2e = ((g2 / np.linalg.norm(W2.astype(np.float64))) * W2).astype(np.float32)

    BF = ml_dtypes.bfloat16
    F8 = ml_dtypes.float8_e4m3   # TRN FP8_EXP4 (max ±240, has inf)
    # W1 is scaled by S1 so its tiny entries survive e4m3; hq/b1 carry the
    # same scale and W2 carries 1/S1 (relu commutes with positive scaling).
    # partition-major repacks: [..., 128 p, chunk, inner]
    w1vt = np.ascontiguousarray(                       # [128, 16, 1024]
        (W1e[:, :VD] * S1).T.reshape(CC, 128, HID).transpose(1, 0, 2)).astype(F8)
    w1qt = (W1e[:, VD:] * S1).T.reshape(QC, 128, HID).transpose(1, 0, 2)  # [128, 8, 1024]
    r = np.arange(R)
    oneh = (np.arange(BK)[:, None] == (r // N)[None, :]).astype(np.float32)
    selb = (np.arange(BL)[None, :] == (np.arange(BK) // K)[:, None]).astype(np.float32)

    packf = np.zeros((128, HID + BL + BK + HID), dtype=np.float32)
    packf[:, 0:HID] = W2e.reshape(1, HID) * (1.0 / S1)
    packf[0:BK, HID:HID + BL] = selb
    packf[0:BL, HID + BL:HID + BL + BK] = selb.T
    packf[0:BK, HID + BL + BK:HID + BL + BK + HID] = b1.reshape(1, HID) * S1

    shared = dict(w1vt=w1vt, oneh=oneh, packf=packf)
    in_maps = []
    for c in range(NCORES):
        vl = v[c * BL:(c + 1) * BL].reshape(R, VD)
        # vt[ch, p, cc, r_in_chunk] = v[ch*RC + r, cc*128 + p]
        vt4 = np.ascontiguousarray(
            vl.T.reshape(CC, 128, NCH, RC).transpose(2, 1, 0, 3)).astype(F8)
        ql = q[c * BL:(c + 1) * BL].reshape(BK, QD)
        qt3 = ql.T.reshape(QC, 128, BK).transpose(1, 0, 2)   # [128, 8, 96]
        qtwq = np.concatenate([qt3, w1qt], axis=2).astype(BF)  # [128, 8, 1120]
        in_maps.append(dict(vth=np.ascontiguousarray(vt4[:2]),
                            vtr=np.ascontiguousarray(vt4[2:]),
                            qtwq=np.ascontiguousarray(qtwq), **shared))
    return in_maps


def kernel(**inputs) -> np.ndarray:
    in_maps = _prepare_in_maps(inputs)
    nc = _get_nc()
    res = run_bass_kernel_spmd(nc, in_maps, list(range(NCORES)))
    outs = [res.results[c]["out"].reshape(BL, K, N, 1) for c in range(NCORES)]
    return np.concatenate(outs, axis=0)



# revision 3
# speedup vs baseline: 1.7508x; 1.0277x over previous
"""Trainium2 Bass kernel for nn_BigAttention (weight-norm MLP + softmax-over-k).

Math (per the reference):
    W1e = g1 * W1 / ||W1||_F          [1024, 3072]
    W2e = g2 * W2 / ||W2||_F          [1, 1024]
    hv  = v @ W1e[:, :2048].T         [B,K,N,1024]
    hq  = q @ W1e[:, 2048:].T         [B,K,1024]
    joint  = relu(hv + hq + b1)
    logits = joint @ W2e.T  (+ b2, which cancels in the softmax over k)
    out = softmax(logits, axis=K)     [B,K,N,1]

Sharding: data-parallel over batch, 8 batches per core; weights replicated.

All heavy matmuls run fp8 e4m3 with perf_mode=DoubleRow: two 128-deep
contraction chunks fuse into one PE instruction streaming 2 cols/cycle
(~2x bf16). W1 is scaled by S1 host-side so its tiny entries survive e4m3;
hq/b1 carry the same scale and W2 carries 1/S1 (relu commutes with scale).

Per-core device program (rows r = (b_local, k, n) flattened, R = 3456):
  - hq[96, 1024] via 16 DoubleRow matmuls in four [48, 512] quadrants
    (bk-half x hid-half); b1 added on the PSUM->SBUF copy; stored fp8 as
    hq8[48, 2, 1024] so the closers can DoubleRow over the two bk halves.
  - main: per 128-row tile, PSUM[row, hidden 1024] accumulates 8 paired
    v^T-chunk DoubleRow matmuls plus two one-hot DoubleRow closers that
    add hq[bk(row), :] (one-hot selection stationary, hq moving).
  - epilogue per tile: one DVE scalar_tensor_tensor computes
    (PSUM max 0) * w2_broadcast with accum_out = per-row sum = the logit.
  - softmax over k: logits go [128, 27] -> StreamTranspose -> linear DRAM ->
    [96 (b,k), 36 n] SBUF; exp on ACT; the per-(b,n) sum and its broadcast
    back over k are two tiny one-hot matmuls on the PE; final scale on DVE;
    one strided DMA writes the [8,12,36,1] output slice.

All heavy inputs are host-repacked "partition-major" so every big DMA is 128
contiguous runs (one per partition). DMA issue order matches consumption
order: v chunk 0 + W1v first (the first matmul's operands), then qtwq,
v chunk 1, constants, one-hots.
"""

import ml_dtypes
import numpy as np

import concourse.bacc as bacc
import concourse.mybir as mybir
import concourse.tile as tile
from concourse.bass_utils import run_bass_kernel_spmd

F32 = mybir.dt.float32
NCORES = 8
B, K, N = 64, 12, 36
VD, QD, HID = 2048, 1024, 1024
BL = B // NCORES              # local batches per core
R = BL * K * N                # 3456 rows per core
BK = BL * K                   # 96 (b,k) groups per core
BH = BK // 2                  # 48: bk-half size for DoubleRow closers
CC = VD // 128                # 16 contraction chunks over v-dim
QC = QD // 128                # 8 contraction chunks over q-dim
RC = 384                      # rows per DMA chunk (9 chunks)
NCH = R // RC
RT = 128                      # rows per PSUM tile
NT = RC // RT
NRT = R // RT                 # 27 row tiles
VSPLIT = 8                    # v-chunk DMA granularity (cc chunks per DMA)

_NC_CACHE = None

MM_DT = mybir.dt.float32r
BF16 = mybir.dt.bfloat16
FP8 = mybir.dt.float8e4
S1 = 2048.0


def _build_nc():
    nc = bacc.Bacc("TRN2", target_bir_lowering=False, debug=False,
                   num_devices=NCORES)

    DR = mybir.MatmulPerfMode.DoubleRow

    def mm(out, lhsT, rhs, **kw):
        nc.tensor.matmul(out, lhsT, rhs, **kw)

    w1vt = nc.dram_tensor("w1vt", [128, CC, HID], FP8, kind="ExternalInput").ap()
    # qt and W1q^T packed along the free dim: [:, cq, 0:96]=q^T, [:, cq, 96:1120]=W1q^T
    qtwq = nc.dram_tensor("qtwq", [128, QC, BK + HID], FP8, kind="ExternalInput").ap()
    # one-hot row-selection for the hq-add closers, split into two 48-bk
    # k-tiles so the closers run DoubleRow: oneh2[p, i, r] = (bk(r) == i*48+p)
    oneh2_d = nc.dram_tensor("oneh2", [BH, 2, R], FP8, kind="ExternalInput").ap()
    # small constants: W2e/S1 replicated (bf16), b1*S1 replicated (bf16),
    # softmax selection matrices (fp32)
    w2b = nc.dram_tensor("w2b", [128, HID], BF16, kind="ExternalInput").ap()
    b1b = nc.dram_tensor("b1b", [BH, HID], BF16, kind="ExternalInput").ap()
    sel = nc.dram_tensor("sel", [BK, BL + BK], F32, kind="ExternalInput").ap()
    # v is split: the first two chunks ride with the weights at the front of
    # the upload order; the bulk uploads last, hidden under early compute.
    vth = nc.dram_tensor("vth", [2, 128, CC, RC], FP8, kind="ExternalInput").ap()
    vtr = nc.dram_tensor("vtr", [NCH - 2, 128, CC, RC], FP8, kind="ExternalInput").ap()
    out = nc.dram_tensor("out", [BL, K, N, 1], F32, kind="ExternalOutput").ap()

    MAX = mybir.AluOpType.max
    MULT = mybir.AluOpType.mult
    BYPASS = mybir.AluOpType.bypass
    ADD = mybir.AluOpType.add

    with tile.TileContext(nc) as tc:
        with tc.tile_pool(name="const", bufs=1) as cpool, \
             tc.tile_pool(name="wv", bufs=1) as wvpool, \
             tc.tile_pool(name="vtp", bufs=2) as vtpool, \
             tc.tile_pool(name="work", bufs=3) as work, \
             tc.tile_pool(name="small", bufs=1) as small, \
             tc.tile_pool(name="dram", bufs=1, space="DRAM") as dpool, \
             tc.tile_pool(name="psum", bufs=4, space="PSUM") as pspool:

            # ---- startup as few fat DMAs (the Tile runtime can only track
            # ~8 outstanding DMA completions; many small DMAs serialize and
            # starve the PE). Issue order matches consumption order: the
            # first matmul pair needs vt chunk 0 + wv group 0.
            def vt_chunk_tiles(ch):
                src_ap = vth[ch] if ch < 2 else vtr[ch - 2]
                tiles = []
                for j in range(CC // VSPLIT):
                    t = vtpool.tile([128, VSPLIT, RC], FP8, tag=f"vt{j}")
                    nc.sync.dma_start(
                        out=t, in_=src_ap[:, j * VSPLIT:(j + 1) * VSPLIT, :])
                    tiles.append(t)
                return tiles

            vt_cur = vt_chunk_tiles(0)

            WG = 4  # wv group size (cc chunks per DMA)
            wv_g = []
            for j in range(CC // WG):
                t = wvpool.tile([128, WG, HID], FP8, tag=f"wvg{j}")
                nc.scalar.dma_start(out=t, in_=w1vt[:, j * WG:(j + 1) * WG, :])
                wv_g.append(t)

            qtwq_s = cpool.tile([128, QC, BK + HID], FP8)
            nc.sync.dma_start(out=qtwq_s, in_=qtwq)

            vt_next = vt_chunk_tiles(1)

            w2b_s = cpool.tile([128, HID], BF16)
            nc.scalar.dma_start(out=w2b_s, in_=w2b)
            b1b_s = cpool.tile([BH, HID], BF16)
            nc.gpsimd.dma_start(out=b1b_s, in_=b1b)
            sel_s = cpool.tile([BK, BL + BK], F32)
            nc.gpsimd.dma_start(out=sel_s, in_=sel)

            oneh2_s = cpool.tile([BH, 2, R], FP8)
            nc.sync.dma_start(out=oneh2_s, in_=oneh2_d)

            selb_s = sel_s[:, 0:BL]
            selbt_s = sel_s[0:BL, BL:BL + BK]

            # per-row logits, laid out [p, rt] with row = rt*128 + p, split
            # into two tiles so the first half's DRAM flush hides under the
            # main loop. 32 columns (StreamTranspose needs 32x32 blocks).
            NRT_A = 18   # 18*128 rows = 64 (b,k) groups — a 32-aligned bk split
            ls_a = cpool.tile([128, 32], F32)
            nc.vector.memset(ls_a, 0.0)
            ls_b = cpool.tile([128, 32], F32)
            nc.vector.memset(ls_b, 0.0)
            lg = dpool.tile([R], F32)
            lg2 = lg.rearrange("(t p) -> t p", t=NRT, p=128)

            def flush_logits(ls, ls_t_name, t0, t1):
                # ls[p, t - t0] holds L[t*128 + p] for t in [t0, t1)
                ls_t = cpool.tile([128, 32], F32, name=ls_t_name)
                nc.vector.transpose(ls_t, ls)
                for i in range(4):
                    eng = nc.sync if i % 2 == 0 else nc.scalar
                    eng.dma_start(
                        out=lg2[t0:t1, 32 * i:32 * i + 32],
                        in_=ls_t[32 * i:32 * i + (t1 - t0), :])

            hq8_s = cpool.tile([BH, 2, HID], FP8)
            s96 = small.tile([BK, N], F32)
            e96 = small.tile([BK, N], F32)
            sums_ps = pspool.tile([BL, N], F32, tag="sm", bufs=2)

            def emit_vmms(t, ps):
                # fp8 DoubleRow: each matmul contracts TWO 128-deep v chunks
                # (lhsT [128, 2, 128 rows], rhs [128, 2, 512]) at 2 cols/cycle.
                for cc in range(0, CC, 2):
                    lhsT = vt_cur[cc // VSPLIT][:, cc % VSPLIT:cc % VSPLIT + 2,
                                                t * RT:(t + 1) * RT]
                    wvc = wv_g[cc // WG][:, cc % WG:cc % WG + 2, :]
                    mm(ps[:, 0:512], lhsT, wvc[:, :, 0:512],
                       start=(cc == 0), stop=False, perf_mode=DR)
                    mm(ps[:, 512:1024], lhsT, wvc[:, :, 512:1024],
                       start=(cc == 0), stop=False, perf_mode=DR)

            def emit_closer(rt, ps):
                oh = oneh2_s[:, :, rt * RT:(rt + 1) * RT]
                mm(ps[:, 0:512], oh, hq8_s[:, :, 0:512],
                   start=False, stop=True, perf_mode=DR)
                mm(ps[:, 512:1024], oh, hq8_s[:, :, 512:1024],
                   start=False, stop=True, perf_mode=DR)
                relu_w2 = work.tile([128, HID], F32, tag="relu_w2")
                ls, col = (ls_a, rt) if rt < NRT_A else (ls_b, rt - NRT_A)
                nc.vector.scalar_tensor_tensor(
                    out=relu_w2, in0=ps, scalar=0.0, in1=w2b_s,
                    op0=MAX, op1=MULT,
                    accum_out=ls[:, col:col + 1])
                if rt == NRT_A - 1:
                    # flush + start the softmax head for bk rows 0:64 while
                    # the main loop still runs
                    flush_logits(ls_a, "ls_ta", 0, NRT_A)
                    nc.sync.dma_start(
                        out=s96[0:64, :],
                        in_=lg.rearrange("(bk n) -> bk n", n=N)[0:64, :])
                    nc.scalar.activation(e96[0:64, :], s96[0:64, :],
                                         mybir.ActivationFunctionType.Exp)
                    mm(sums_ps, selb_s[0:64, :], e96[0:64, :],
                       start=True, stop=False)

            # ---- chunk 0: v-matmuls for tiles 0..2 first, then hq (its DMAs
            # arrive under the v work), then the deferred closers.
            ps0 = []
            for t in range(NT):
                ps = pspool.tile([128, HID], F32, tag="ps", bufs=3)
                emit_vmms(t, ps)
                ps0.append(ps)

            # hq in four [48, 512] quadrants (bk-half h x hid-half hh), all
            # fp8 DoubleRow over q-chunk pairs; b1 (scaled) added on the
            # PSUM -> SBUF copy, output packed fp8 as hq8[48, 2, 1024].
            for h in range(2):
                for hh in range(2):
                    ps_q = pspool.tile([BH, 512], F32, tag="sm", bufs=2,
                                       name=f"hq_ps{h}{hh}")
                    for cq in range(0, QC, 2):
                        mm(ps_q,
                           qtwq_s[:, cq:cq + 2, h * BH:(h + 1) * BH],
                           qtwq_s[:, cq:cq + 2,
                                  BK + hh * 512:BK + (hh + 1) * 512],
                           start=(cq == 0), stop=(cq == QC - 2), perf_mode=DR)
                    nc.vector.scalar_tensor_tensor(
                        out=hq8_s[:, h, hh * 512:(hh + 1) * 512],
                        in0=ps_q, scalar=0.0,
                        in1=b1b_s[:, hh * 512:(hh + 1) * 512],
                        op0=BYPASS, op1=ADD)

            for t in range(NT):
                emit_closer(t, ps0[t])
            vt_cur = vt_next

            # ---- chunks 1..8
            for ch in range(1, NCH):
                if ch + 1 < NCH:
                    vt_next = vt_chunk_tiles(ch + 1)
                for t in range(NT):
                    rt = ch * NT + t
                    ps = pspool.tile([128, HID], F32, tag="ps", bufs=3)
                    emit_vmms(t, ps)
                    emit_closer(rt, ps)
                vt_cur = vt_next

            # ---- flush remaining logits, finish the softmax
            flush_logits(ls_b, "ls_tb", NRT_A, NRT)
            nc.sync.dma_start(
                out=s96[64:BK, :],
                in_=lg.rearrange("(bk n) -> bk n", n=N)[64:BK, :])
            nc.scalar.activation(e96[64:BK, :], s96[64:BK, :],
                                 mybir.ActivationFunctionType.Exp)
            mm(sums_ps, selb_s[64:BK, :], e96[64:BK, :],
               start=False, stop=True)
            rcp = small.tile([BL, N], F32)
            nc.vector.reciprocal(rcp, sums_ps)
            rexp_ps = pspool.tile([BK, N], F32, tag="sm", bufs=2)
            mm(rexp_ps, selbt_s, rcp, start=True, stop=True)
            w96 = small.tile([BK, N], F32)
            nc.vector.scalar_tensor_tensor(
                out=w96, in0=e96, scalar=0.0, in1=rexp_ps,
                op0=BYPASS, op1=MULT)
            nc.sync.dma_start(
                out=out.rearrange("b k n o -> (b k) (n o)"), in_=w96)

    nc.compile()
    return nc


def _get_nc():
    global _NC_CACHE
    if _NC_CACHE is None:
        _NC_CACHE = _build_nc()
    return _NC_CACHE


def _prepare_in_maps(inputs):
    v = np.asarray(inputs["v"], dtype=np.float32)
    q = np.asarray(inputs["q"], dtype=np.float32)
    W1 = np.asarray(inputs["W1"], dtype=np.float32)
    g1 = np.float64(np.asarray(inputs["g1"]))
    b1 = np.asarray(inputs["b1"], dtype=np.float32)
    W2 = np.asarray(inputs["W2"], dtype=np.float32)
    g2 = np.float64(np.asarray(inputs["g2"]))
    # b2 is a scalar added to every logit -> cancels in softmax over k.

    W1e = ((g1 / np.linalg.norm(W1.astype(np.float64))) * W1).astype(np.float32)
    W2e = ((g2 / np.linalg.norm(W2.astype(np.float64))) * W2).astype(np.float32)

    BF = ml_dtypes.bfloat16
    F8 = ml_dtypes.float8_e4m3   # TRN FP8_EXP4 (max ±240, has inf)
    # partition-major repacks: [..., 128 p, chunk, inner]
    w1vt = np.ascontiguousarray(                       # [128, 16, 1024]
        (W1e[:, :VD] * S1).T.reshape(CC, 128, HID).transpose(1, 0, 2)).astype(F8)
    w1qt = (W1e[:, VD:] * S1).T.reshape(QC, 128, HID).transpose(1, 0, 2)  # [128, 8, 1024]
    r = np.arange(R)
    bk_of_r = r // N
    # oneh2[p, i, r] = 1 iff bk(r) == i*48 + p
    oneh2 = np.zeros((BH, 2, R), dtype=np.float32)
    oneh2[bk_of_r % BH, bk_of_r // BH, r] = 1.0
    selb = (np.arange(BL)[None, :] == (np.arange(BK) // K)[:, None]).astype(np.float32)

    w2bf = np.broadcast_to((W2e.reshape(1, HID) * (1.0 / S1)), (128, HID))
    b1bf = np.broadcast_to((b1.reshape(1, HID) * S1), (BH, HID))
    sel = np.zeros((BK, BL + BK), dtype=np.float32)
    sel[:, 0:BL] = selb
    sel[0:BL, BL:BL + BK] = selb.T

    shared = dict(w1vt=w1vt, oneh2=oneh2.astype(F8),
                  w2b=np.ascontiguousarray(w2bf).astype(BF),
                  b1b=np.ascontiguousarray(b1bf).astype(BF), sel=sel)
    in_maps = []
    for c in range(NCORES):
        vl = v[c * BL:(c + 1) * BL].reshape(R, VD)
        # vt[ch, p, cc, r_in_chunk] = v[ch*RC + r, cc*128 + p]
        vt4 = np.ascontiguousarray(
            vl.T.reshape(CC, 128, NCH, RC).transpose(2, 1, 0, 3)).astype(F8)
        ql = q[c * BL:(c + 1) * BL].reshape(BK, QD)
        qt3 = ql.T.reshape(QC, 128, BK).transpose(1, 0, 2)   # [128, 8, 96]
        qtwq = np.concatenate([qt3, w1qt], axis=2)           # [128, 8, 1120]
        in_maps.append(dict(vth=np.ascontiguousarray(vt4[:2]),
                            vtr=np.ascontiguousarray(vt4[2:]),
                            qtwq=np.ascontiguousarray(qtwq).astype(F8),
                            **shared))
    return in_maps


def kernel(**inputs) -> np.ndarray:
    in_maps = _prepare_in_maps(inputs)
    nc = _get_nc()
    res = run_bass_kernel_spmd(nc, in_maps, list(range(NCORES)))
    outs = [res.results[c]["out"].reshape(BL, K, N, 1) for c in range(NCORES)]
    return np.concatenate(outs, axis=0)
